# revision 2
# baseline (speedup 1.0000x reference)
"""ChannelAttention (Softmax2d-over-batch) Trainium2 kernel, 8-core SPMD, v2.

Data-parallel over batch (4 samples/core); the only collective is a bf16
AllReduce of S = sum_local_b exp(scores_b - SHIFT)  (3.28 MB).

v2 vs baseline:
  - all GEMM operands bf16 (1 cyc/row on PE, half the SBUF/DMA bytes)
  - E = exp(scoresT) stays RESIDENT in SBUF (kills the 26 MB/core DRAM
    round-trip); V bounces through DRAM in bf16 (2.6 MB each way)
  - scoresT accumulated into a 3-bank [P,1536] PSUM tile -> ONE wide exp
  - AllReduce in bf16, split into 5 chunks issued as phase B finalizes each
    pair of d-chunks, so the collective pipelines behind B + the V GEMMs
  - per-partition biases (bv, alpha*br) folded into ACT/DVE ops; only the
    K/Q biases (which vary along the free dim) keep the rank-1 matmul trick
  - E *= 1/Z done in place; E slices feed the att GEMM directly

SBUF high-water (KB/partition, 208 usable): A 177, B 191, C1 152, C2/3 182.
"""

import numpy as np
import ml_dtypes

AR_SPLIT = 5  # AllReduce pipelined in 5 chunks behind phase B

import concourse.bass as bass
import concourse.tile as tile
from concourse import bacc, mybir
from concourse import bass_utils

B, C, S, HW = 32, 1280, 16, 256
P = 128
KC = C // P          # 10 chunks of the channel dim
NCORES = 8
BL = B // NCORES     # 4 samples per core
SHIFT = 45.0
CGROUPS = [(0, 512), (512, 512), (1024, 256)]  # psum-bank-sized column groups
F32 = mybir.dt.float32
BF16 = mybir.dt.bfloat16
AF = mybir.ActivationFunctionType
BF16NP = ml_dtypes.bfloat16

_CACHE = {}


def _emit(nc, tc, io, alpha):
    ones, brow = io["ones_t"], io["brow"]      # ones [1,P] bf16; brow bk/bq [1,C] bf16
    bv_sb, bra_sb = io["bv_sb"], io["bra_sb"]  # [P, KC] f32 per-partition biases
    xb_d, xf_d = io["xb_d"], io["xf_d"]
    wk_d, wq_d, wv_d, wr_d = io["wk_d"], io["wq_d"], io["wv_d"], io["wr_d"]
    v_d, s_in, s_out, out_d = io["v_d"], io["s_in"], io["s_out"], io["out_d"]

    with tc.tile_pool(name="epool", bufs=1) as epool:      # 100 KB/p, B..C2
        e_sb = epool.tile([P, BL, KC, C], BF16, tag="E")

        with tc.tile_pool(name="xpool", bufs=1) as xpool:  # 20 KB/p, A..C1
            x_sb = xpool.tile([P, BL, KC, HW], BF16, tag="x")
            for b in range(BL):
                nc.sync.dma_start(
                    x_sb[:, b], xb_d.ap()[b].rearrange("(k p) n -> p k n", p=P)
                )

            # ===== phase A: Kt, Qt resident in SBUF (bf16) =====
            with tc.tile_pool(name="ktqt", bufs=1) as ktqtp:   # 40 KB/p
                kt_sb = ktqtp.tile([P, 2, BL, C], BF16, tag="kt")
                qt_sb = ktqtp.tile([P, 2, BL, C], BF16, tag="qt")
                with (
                    tc.tile_pool(name="wA", bufs=12) as wpA,   # 12 KB/p
                    tc.tile_pool(name="psumA", bufs=3, space="PSUM") as psA,
                ):
                    for wd, bias, dest in ((wk_d, "bk", kt_sb), (wq_d, "bq", qt_sb)):
                        for cgs, cgl in CGROUPS:
                            wt = []
                            for k in range(KC):
                                t = wpA.tile([P, 512], BF16, tag="wA")
                                nc.sync.dma_start(
                                    t[:, :cgl],
                                    wd.ap()[k * P:(k + 1) * P, cgs:cgs + cgl],
                                )
                                wt.append(t)
                            for b in range(BL):
                                for hwt in range(2):
                                    ps = psA.tile([P, 512], F32, tag="psA")
                                    for k in range(KC):
                                        nc.tensor.matmul(
                                            ps[:, :cgl],
                                            x_sb[:, b, k, hwt * P:(hwt + 1) * P],
                                            wt[k][:, :cgl],
                                            start=(k == 0),
                                            stop=False,
                                        )
                                    nc.tensor.matmul(
                                        ps[:, :cgl],
                                        ones[:, :P],
                                        brow[bias][:, cgs:cgs + cgl],
                                        start=False,
                                        stop=True,
                                    )
                                    nc.scalar.copy(
                                        dest[:, hwt, b, cgs:cgs + cgl], ps[:, :cgl]
                                    )

                # ===== phase B (dt-outer): scoresT -> wide exp -> E; S += E;
                # s_in chunk written as each dt finalizes; AllReduce split in
                # two halves so the first starts mid-B =====
                with (
                    tc.tile_pool(name="spool", bufs=1) as spool,   # 25.6 KB/p
                    tc.tile_pool(name="psumB", bufs=2, space="PSUM") as psB,
                ):
                    s_sb = spool.tile([P, KC, C], BF16, tag="S")
                    for dt_ in range(KC):
                        for b in range(BL):
                            ps = psB.tile([P, 1536], F32, tag="psB")
                            for cgs, cgl in CGROUPS:
                                for hwt in range(2):
                                    nc.tensor.matmul(
                                        ps[:, cgs:cgs + cgl],
                                        qt_sb[:, hwt, b, dt_ * P:(dt_ + 1) * P],
                                        kt_sb[:, hwt, b, cgs:cgs + cgl],
                                        start=(hwt == 0),
                                        stop=(hwt == 1),
                                    )
                            nc.scalar.activation(
                                e_sb[:, b, dt_], ps[:, :C], AF.Exp,
                                bias=-SHIFT, scale=1.0,
                            )
                            if b == 0:
                                nc.vector.tensor_copy(s_sb[:, dt_], e_sb[:, b, dt_])
                            else:
                                nc.vector.tensor_add(
                                    s_sb[:, dt_], s_sb[:, dt_], e_sb[:, b, dt_]
                                )
                        nc.sync.dma_start(s_in.ap()[dt_], s_sb[:, dt_])
                        seg = KC // AR_SPLIT
                        if dt_ % seg == seg - 1 and dt_ < KC - 1:
                            nc.gpsimd.collective_compute(
                                "AllReduce",
                                mybir.AluOpType.add,
                                replica_groups=[list(range(NCORES))],
                                ins=[s_in.ap()[dt_ + 1 - seg:dt_ + 1]],
                                outs=[s_out.ap()[dt_ + 1 - seg:dt_ + 1]],
                            )

            seg = KC // AR_SPLIT
            nc.gpsimd.collective_compute(
                "AllReduce",
                mybir.AluOpType.add,
                replica_groups=[list(range(NCORES))],
                ins=[s_in.ap()[KC - seg:KC]],
                outs=[s_out.ap()[KC - seg:KC]],
            )

            # ===== phase C1: V -> DRAM bf16 (overlaps the AllReduce) =====
            with (
                tc.tile_pool(name="wV", bufs=1) as wpV,      # 25.6 KB/p
                tc.tile_pool(name="vout", bufs=2) as voutp,  # 10 KB/p
                tc.tile_pool(name="psumV", bufs=3, space="PSUM") as psV,
            ):
                wv_sb = wpV.tile([P, KC, C], BF16, tag="wv")
                nc.sync.dma_start(
                    wv_sb[:], wv_d.ap().rearrange("(k p) n -> p k n", p=P)
                )
                for b in range(BL):
                    vt = voutp.tile([P, KC, HW], BF16, tag="Vout")
                    for vct in range(KC):
                        ps = psV.tile([P, HW], F32, tag="psV")
                        for ci in range(KC):
                            nc.tensor.matmul(
                                ps[:],
                                wv_sb[:, ci, vct * P:(vct + 1) * P],
                                x_sb[:, b, ci],
                                start=(ci == 0),
                                stop=(ci == KC - 1),
                            )
                        nc.scalar.activation(
                            vt[:, vct], ps[:], AF.Identity,
                            bias=bv_sb[:, vct:vct + 1], scale=1.0,
                        )
                    nc.sync.dma_start(
                        v_d.ap()[b].rearrange("k p n -> p k n"), vt[:]
                    )

        # ============ phases R + C2 + C3 ============
        with (
            tc.tile_pool(name="rpool", bufs=1) as rpool,     # 25.6 KB/p
            tc.tile_pool(name="zbuf", bufs=1) as zbufp,
            tc.tile_pool(name="vload", bufs=2) as vlp,       # 10 KB/p
            tc.tile_pool(name="attp", bufs=2) as attp,       # 10 KB/p
            tc.tile_pool(name="wrfull", bufs=1) as wrp,      # 25.6 KB/p
            tc.tile_pool(name="fin", bufs=6) as finp,        # 6 KB/p
            tc.tile_pool(name="psumC", bufs=3, space="PSUM") as psC,
            tc.tile_pool(name="psumC2", bufs=5, space="PSUM") as psA2,
        ):
            r_sb = rpool.tile([P, KC, C], BF16, tag="R")
            wr_sb = wrp.tile([P, KC, C], BF16, tag="wrt")
            nc.sync.dma_start(
                wr_sb[:], wr_d.ap().rearrange("(k p) n -> p k n", p=P)
            )
            for dt_ in range(KC):
                zt = zbufp.tile([P, C], BF16, tag="Zb")
                nc.sync.dma_start(zt[:], s_out.ap()[dt_])
                zf = zbufp.tile([P, C], F32, tag="Zf")
                # +1e-30: keep Z away from 0/denormals (reciprocal_approx
                # is undefined there)
                nc.scalar.add(zf[:], zt[:], 1e-30)
                rf = zbufp.tile([P, C], F32, tag="Rf")
                nc.vector.reciprocal_approx_fast(rf[:], zf[:])
                nc.scalar.copy(r_sb[:, dt_], rf[:])

            for b in range(BL):
                v_sb = vlp.tile([P, KC, HW], BF16, tag="Vl")
                nc.sync.dma_start(
                    v_sb[:], v_d.ap()[b].rearrange("k p n -> p k n")
                )
                att_sb = attp.tile([P, KC, HW], BF16, tag="att")
                # dt-outer in two waves of 5 output chunks; each accumulator
                # owns a full PSUM bank (concurrent accumulation chains must
                # not share a bank on HW). E*=R (wave 1) pipelines with the
                # matmuls chunk-by-chunk.
                for wave in range(2):
                    aps = []
                    for api in range(KC // 2):
                        ap_t = psA2.tile([P, 512], F32, tag="psATT")
                        aps.append(ap_t)
                    for dt_ in range(KC):
                        if wave == 0:
                            nc.vector.tensor_mul(
                                e_sb[:, b, dt_], e_sb[:, b, dt_], r_sb[:, dt_]
                            )
                        for ci in range(KC // 2):
                            ct = wave * (KC // 2) + ci
                            nc.tensor.matmul(
                                aps[ci][:, :HW],
                                e_sb[:, b, dt_, ct * P:(ct + 1) * P],
                                v_sb[:, dt_],
                                start=(dt_ == 0),
                                stop=(dt_ == KC - 1),
                            )
                    for ci in range(KC // 2):
                        ct = wave * (KC // 2) + ci
                        nc.scalar.copy(att_sb[:, ct], aps[ci][:, :HW])
                for ot in range(KC):
                    ps_full = psC.tile([P, 512], F32, tag="psREF")
                    for ct in range(KC):
                        nc.tensor.matmul(
                            ps_full[:, :HW],
                            wr_sb[:, ct, ot * P:(ot + 1) * P],
                            att_sb[:, ct],
                            start=(ct == 0),
                            stop=(ct == KC - 1),
                        )
                    xt = finp.tile([P, HW], F32, tag="xload")
                    nc.sync.dma_start(xt[:], xf_d.ap()[b, ot * P:(ot + 1) * P, :])
                    ot_t = finp.tile([P, HW], F32, tag="outT")
                    # out = (alpha * psum + alpha*br) + x
                    nc.vector.affine_then_add(
                        ot_t[:], ps_full[:, :HW], xt[:],
                        scale=alpha, bias=bra_sb[:, ot:ot + 1],
                    )
                    nc.sync.dma_start(
                        out_d.ap()[b, ot * P:(ot + 1) * P, :], ot_t[:]
                    )


def build(alpha: float, nrep: int = 1):
    nc = bacc.Bacc(
        "TRN2",
        target_bir_lowering=False,
        debug=False,
        enable_asserts=False,
        num_devices=NCORES,
    )

    io = {}
    io["xb_d"] = nc.dram_tensor("xb", [BL, C, HW], BF16, kind="ExternalInput")
    io["xf_d"] = nc.dram_tensor("xf", [BL, C, HW], F32, kind="ExternalInput")
    io["wk_d"] = nc.dram_tensor("wkt", [C, C], BF16, kind="ExternalInput")  # Wk.T
    io["wq_d"] = nc.dram_tensor("wqt", [C, C], BF16, kind="ExternalInput")
    io["wv_d"] = nc.dram_tensor("wvt", [C, C], BF16, kind="ExternalInput")
    io["wr_d"] = nc.dram_tensor("wrt", [C, C], BF16, kind="ExternalInput")
    for nm in ("bk", "bq"):
        io[nm] = nc.dram_tensor(nm, [1, C], BF16, kind="ExternalInput")
    io["bv"] = nc.dram_tensor("bv", [C], F32, kind="ExternalInput")
    io["bra"] = nc.dram_tensor("bra", [C], F32, kind="ExternalInput")  # alpha*br
    io["ones_d"] = nc.dram_tensor("ones", [1, P], BF16, kind="ExternalInput")
    io["out_d"] = nc.dram_tensor("out", [BL, C, HW], F32, kind="ExternalOutput")

    io["v_d"] = nc.dram_tensor("v_scr", [BL, KC, P, HW], BF16)
    io["s_in"] = nc.dram_tensor("s_in", [KC, P, C], BF16)
    io["s_out"] = nc.dram_tensor("s_out", [KC, P, C], BF16, addr_space="Shared")

    # const AP so ACT Exp can take bias=-SHIFT
    cshift = nc.alloc_sbuf_tensor("const-shift", [128, 1], F32)
    nc.gpsimd.memset(cshift.ap(), -SHIFT)
    nc.const_aps.aps[(F32, -SHIFT)] = cshift.ap()
    ceps = nc.alloc_sbuf_tensor("const-eps", [128, 1], F32)
    nc.gpsimd.memset(ceps.ap(), 1e-30)
    nc.const_aps.aps[(F32, 1e-30)] = ceps.ap()
    nc.all_engine_barrier()

    with tile.TileContext(nc) as tc:
        with tc.tile_pool(name="cpool", bufs=1) as cpool:
            ones = cpool.tile([1, P], BF16, tag="ones")
            nc.sync.dma_start(ones[:], io["ones_d"].ap())
            brow = {}
            for nm in ("bk", "bq"):
                t = cpool.tile([1, C], BF16, tag=f"row_{nm}")
                nc.sync.dma_start(t[:], io[nm].ap())
                brow[nm] = t
            bv_sb = cpool.tile([P, KC], F32, tag="bv")
            nc.sync.dma_start(bv_sb[:], io["bv"].ap().rearrange("(k p) -> p k", p=P))
            bra_sb = cpool.tile([P, KC], F32, tag="bra")
            nc.sync.dma_start(bra_sb[:], io["bra"].ap().rearrange("(k p) -> p k", p=P))
            io["ones_t"] = ones
            io["brow"] = brow
            io["bv_sb"] = bv_sb
            io["bra_sb"] = bra_sb

            for _ in range(nrep):
                _emit(nc, tc, io, alpha)

    nc.compile()
    return nc


def make_in_maps(x, Wq, bq, Wk, bk, Wv, bv, Wr, br, alpha):
    alpha_f = float(np.asarray(alpha).reshape(-1)[0])
    xs = np.ascontiguousarray(np.asarray(x, dtype=np.float32).reshape(B, C, HW))
    xb = xs.astype(BF16NP)
    w = {
        "wkt": np.ascontiguousarray(np.asarray(Wk, np.float32).T).astype(BF16NP),
        "wqt": np.ascontiguousarray(np.asarray(Wq, np.float32).T).astype(BF16NP),
        "wvt": np.ascontiguousarray(np.asarray(Wv, np.float32).T).astype(BF16NP),
        "wrt": np.ascontiguousarray(np.asarray(Wr, np.float32).T).astype(BF16NP),
    }
    rows = {
        "bk": np.asarray(bk, np.float32).reshape(1, C).astype(BF16NP),
        "bq": np.asarray(bq, np.float32).reshape(1, C).astype(BF16NP),
        "bv": np.ascontiguousarray(np.asarray(bv, np.float32).reshape(C)),
        "bra": np.ascontiguousarray(
            (alpha_f * np.asarray(br, np.float32)).reshape(C)
        ),
    }
    in_maps = []
    for c in range(NCORES):
        sl = slice(c * BL, (c + 1) * BL)
        in_maps.append({
            "xb": np.ascontiguousarray(xb[sl]),
            "xf": np.ascontiguousarray(xs[sl]),
            "ones": np.ones((1, P), dtype=BF16NP),
            **w, **rows,
        })
    return in_maps


def kernel(x, Wq, bq, Wk, bk, Wv, bv, Wr, br, alpha):
    alpha_f = float(np.asarray(alpha).reshape(-1)[0])
    key = ("v2", alpha_f)
    if key not in _CACHE:
        _CACHE[key] = build(alpha_f)
    nc = _CACHE[key]
    in_maps = make_in_maps(x, Wq, bq, Wk, bk, Wv, bv, Wr, br, alpha)
    res = bass_utils.run_bass_kernel_spmd(nc, in_maps, core_ids=list(range(NCORES)))
    out = np.concatenate([res.results[c]["out"] for c in range(NCORES)], axis=0)
    return np.ascontiguousarray(out.reshape(B, C, S, S).astype(np.float32))
